# revision 1
# baseline (speedup 1.0000x reference)
"""Causal self-attention (B=4, T=2048, E=1024, H=16, D=64) on 8 trn2 cores.

Sharding: core c -> (batch b = c//2, head-group g = c%2 of 8 heads).
Each core computes qkv projection + RoPE + causal attention + its partial
output projection for its (batch, head-group); host sums the two
head-group partials per batch and transposes back.

Device data layout is feature-major ("T" suffix = [features, tokens]):
scores are computed k-major (S.T blocks [tk=128, tq]) so causal masking
skips ~half the matmuls, and softmax normalization comes from an extra
ones-column in the v operand of the PV matmul (the denominator lands in
one PSUM partition row at zero extra matmul cost).

All matmuls run in float32r (full PE rate for N>=256). The BIR verifier
requires float32r matmul operands to be *produced* as float32r, so every
matmul-feeding tile is declared float32r; engine inputs are read via
.bitcast(float32) where needed (same bits).

Scheduling (v4):
- qkv chunks and attention q-chunks are interleaved: after the qkv
  chunk covering q columns [512*qq, 512*qq+512) finishes, the attention
  for q-chunk qq of both pairs runs, so x/weight DMA streams behind
  attention compute and the tensor engine never waits on HBM.
- the first qkv chunk of group 0 is split 256+256 so the first matmul
  only needs 1.5 MB of DMA; cos/ssin stream in 512-col pieces; group-1
  weights, w_proj and x prefetch during group-0 attention.
- RoPE's pair-swap permutation runs on the DVE via stream_shuffle
  (no perm matmul, no extra PSUM bank).
- PV matmuls run one k-tile behind the score matmuls so exp is off the
  tensor critical path; the output projection runs one q-chunk behind.
- softmax drain: psum -> SBUF copy (frees the bank), denominator row
  broadcast via two cross-quadrant stream_shuffles + one 64-wide fast
  reciprocal, head-1 rows placed with cross-quadrant DVE writes. No
  DMA, no gpsimd custom ops (gpsimd only does mask muls + ones DMAs).
"""
import sys

sys.path.insert(0, "/opt/trn_rl_repo")

from contextlib import ExitStack

import numpy as np

import concourse.bass as bass
import concourse.bacc as bacc
import concourse.tile as tile
from concourse import mybir
from concourse.bass_utils import run_bass_kernel_spmd

B, T, E, H, D = 4, 2048, 1024, 16, 64
NCORES = 8
HG = H // 2          # heads per shard (8)
F = HG * D           # features per shard (512)
NPAIR = F // 128     # head pairs per shard (4)
NGRP = NPAIR // 2    # pair groups (2)
KE = E // 128        # contraction tiles over E (8)
NKT = T // 128       # k tiles (16)
F32 = mybir.dt.float32
F32R = mybir.dt.float32r
EXP = mybir.ActivationFunctionType.Exp
SWAP_MASK = [i ^ 1 for i in range(32)]   # rope pair swap within banks
BCAST_MASK = [0] * 32                     # all lanes take lane 0

_NC_CACHE = {}


def _build_program():
    if "nc" in _NC_CACHE:
        return _NC_CACHE["nc"]
    nc = bacc.Bacc("TRN2", target_bir_lowering=False, debug=False,
                   num_devices=NCORES)
    mm = nc.tensor.matmul
    xT = nc.dram_tensor("xT", [E, T], F32R, kind="ExternalInput").ap()
    wqT = nc.dram_tensor("wqT", [E, F], F32R, kind="ExternalInput").ap()
    wkT = nc.dram_tensor("wkT", [E, F], F32R, kind="ExternalInput").ap()
    wvT = nc.dram_tensor("wvT", [E, F], F32R, kind="ExternalInput").ap()
    wpT = nc.dram_tensor("wpT", [F, E], F32R, kind="ExternalInput").ap()
    cos2 = nc.dram_tensor("cos2", [128, T], F32, kind="ExternalInput").ap()
    ssp2 = nc.dram_tensor("ssp2", [128, T], F32, kind="ExternalInput").ap()
    maskd = nc.dram_tensor("maskd", [128, 128], F32, kind="ExternalInput").ap()
    ones16 = nc.dram_tensor("ones16", [128, NKT], F32R,
                            kind="ExternalInput").ap()
    outT = nc.dram_tensor("outT", [E, T], F32, kind="ExternalOutput").ap()

    xT_r = xT.rearrange("(ke p) t -> p ke t", p=128)
    wq_r = wqT.rearrange("(ke p) j -> p ke j", p=128)
    wk_r = wkT.rearrange("(ke p) j -> p ke j", p=128)
    wv_r = wvT.rearrange("(ke p) j -> p ke j", p=128)
    wp_r = wpT.rearrange("(kf p) o -> p kf o", p=128)

    with tile.TileContext(nc) as tc:
        with ExitStack() as ctx:
            const = ctx.enter_context(tc.tile_pool(name="const", bufs=1))
            wgp = ctx.enter_context(tc.tile_pool(name="wgp", bufs=1))
            xp = ctx.enter_context(tc.tile_pool(name="xp", bufs=3))
            qkp = ctx.enter_context(tc.tile_pool(name="qkp", bufs=2))
            vp = ctx.enter_context(tc.tile_pool(name="vp", bufs=3))
            yp = ctx.enter_context(tc.tile_pool(name="yp", bufs=4))
            pp = ctx.enter_context(tc.tile_pool(name="pp", bufs=4))
            tmp = ctx.enter_context(tc.tile_pool(name="tmp", bufs=2))
            ysbp = ctx.enter_context(tc.tile_pool(name="ysbp", bufs=3))
            bcp = ctx.enter_context(tc.tile_pool(name="bcp", bufs=2))
            outp = ctx.enter_context(tc.tile_pool(name="outp", bufs=2))
            psA = ctx.enter_context(
                tc.tile_pool(name="psA", bufs=2, space="PSUM"))
            psS = ctx.enter_context(
                tc.tile_pool(name="psS", bufs=2, space="PSUM"))
            psY = ctx.enter_context(
                tc.tile_pool(name="psY", bufs=2, space="PSUM"))

            c_cos = const.tile([128, T], F32, tag="cos")
            c_ssp = const.tile([128, T], F32, tag="ssp")
            c_mask = const.tile([128, 128], F32, tag="mask")
            c_wp = const.tile([128, NPAIR, E], F32R, tag="wp")

            def load_wg(g):
                wg = wgp.tile([128, KE, 768], F32R, tag="wg", name=f"wg{g}")
                j0 = 256 * g
                nc.sync.dma_start(out=wg[:, :, 0:256],
                                  in_=wq_r[:, :, j0:j0 + 256])
                return wg, j0

            def load_x(tcs, tch):
                xca = xp.tile([128, KE // 2, tch], F32R, tag="xc",
                              name=f"xca{tcs}")
                xcb = xp.tile([128, KE // 2, tch], F32R, tag="xc",
                              name=f"xcb{tcs}")
                nc.sync.dma_start(out=xca,
                                  in_=xT_r[:, 0:KE // 2, tcs:tcs + tch])
                nc.sync.dma_start(out=xcb,
                                  in_=xT_r[:, KE // 2:KE, tcs:tcs + tch])
                return xca, xcb

            # ---- startup DMA order: first-needed bytes first ----
            wg0, j0g0 = load_wg(0)           # wq group 0 (1 MB)
            x_pre = load_x(0, 256)           # first 256-col x chunk (1 MB)
            nc.sync.dma_start(out=wg0[:, :, 256:512],
                              in_=wk_r[:, :, j0g0:j0g0 + 256])
            nc.sync.dma_start(out=c_cos[:, 0:512], in_=cos2[:, 0:512])
            nc.sync.dma_start(out=c_ssp[:, 0:512], in_=ssp2[:, 0:512])
            nc.sync.dma_start(out=wg0[:, :, 512:768],
                              in_=wv_r[:, :, j0g0:j0g0 + 256])
            for cc in range(1, 4):
                nc.sync.dma_start(out=c_cos[:, 512 * cc:512 * cc + 512],
                                  in_=cos2[:, 512 * cc:512 * cc + 512])
                nc.sync.dma_start(out=c_ssp[:, 512 * cc:512 * cc + 512],
                                  in_=ssp2[:, 512 * cc:512 * cc + 512])
            nc.sync.dma_start(out=c_mask, in_=maskd)

            y_tiles = [None] * NPAIR
            wg = wg0

            def attention_chunk(p, qq, qT, kT, v3, yT):
                qb = 512 * qq
                kts = list(range(4 * qq + 4))
                last = kts[-1]
                psy0 = psY.tile([128, 512], F32, tag="psY", name="psy0")
                psy1 = psY.tile([128, 512], F32, tag="psY", name="psy1")
                psy = (psy0, psy1)

                def pv(item):
                    kt, pt, col_lo = item
                    for hl in range(2):
                        mm(psy[hl][0:65, col_lo:512],
                           v3[:, kt, 65 * hl:65 * hl + 65],
                           pt[:, 512 * hl + col_lo:512 * hl + 512],
                           start=(kt == 0), stop=(kt == last),
                           skip_group_check=True)

                prev = None
                for kt in kts:
                    col_lo = max(qb, 128 * kt) - qb
                    pS = psS.tile([128, 1024], F32, tag="psS")
                    for hl in range(2):
                        hr = 64 * hl
                        mm(pS[:, 512 * hl + col_lo:512 * hl + 512],
                           kT[hr:hr + 64, 128 * kt:128 * kt + 128],
                           qT[hr:hr + 64, qb + col_lo:qb + 512],
                           start=True, stop=True, skip_group_check=True)
                    pt = pp.tile([128, 1024], F32R, tag="pt")
                    pS2 = pS.rearrange("p (h c) -> p h c", h=2)
                    pt2 = pt.rearrange("p (h c) -> p h c", h=2)
                    nc.scalar.activation(
                        pt2[:, :, col_lo:512], pS2[:, :, col_lo:512],
                        EXP, scale=0.125)
                    if 128 * kt >= qb:  # diagonal block, both heads
                        for hl in range(2):
                            o = 512 * hl + col_lo
                            nc.gpsimd.tensor_mul(
                                pt[:, o:o + 128],
                                pt[:, o:o + 128].bitcast(F32), c_mask)
                    if prev is not None:
                        pv(prev)
                    prev = (kt, pt, col_lo)
                pv(prev)

                if p == NPAIR - 1 and qq > 0:
                    _proj(nc, mm, psA, outp, c_wp, y_tiles, outT, qq - 1)
                # ---- softmax drain (scalar + vector only) ----
                for hl in range(2):
                    y_sb = ysbp.tile([96, 512], F32, tag="ysb")
                    nc.scalar.copy(y_sb[0:65, :], psy[hl][0:65, :])
                    bcr = bcp.tile([64, 512], F32, tag="bcr")
                    nc.vector.stream_shuffle(
                        bcr[0:32, :], y_sb[64:96, :], BCAST_MASK)
                    nc.vector.stream_shuffle(
                        bcr[32:64, :], y_sb[64:96, :], BCAST_MASK)
                    bc = bcp.tile([64, 512], F32, tag="bc")
                    nc.vector.reciprocal_approx_fast(bc, bcr)
                    col = slice(qb, qb + 512)
                    if hl == 0:
                        nc.vector.tensor_mul(
                            yT[0:64, col], y_sb[0:64, :], bc)
                    else:
                        nc.vector.tensor_mul(
                            yT[64:96, col], y_sb[0:32, :], bc[0:32, :])
                        nc.vector.tensor_mul(
                            yT[96:128, col], y_sb[32:64, :], bc[32:64, :])

            for g in range(NGRP):
                pair_qk = []
                pair_v = []
                pair_y = []
                for pi in range(2):
                    qT = qkp.tile([128, T], F32R, tag="qT")
                    kT = qkp.tile([128, T], F32R, tag="kT")
                    v3 = vp.tile([128, NKT, 130], F32R, tag="v3")
                    nc.gpsimd.dma_start(out=v3[:, :, 64], in_=ones16)
                    nc.gpsimd.dma_start(out=v3[:, :, 129], in_=ones16)
                    yT = yp.tile([128, T], F32R, tag="yT")
                    pair_qk.append((qT, kT))
                    pair_v.append(v3)
                    pair_y.append(yT)
                    y_tiles[2 * g + pi] = yT

                chunks = ([(0, 256), (256, 256), (512, 512),
                           (1024, 512), (1536, 512)] if g == 0 else
                          [(0, 512), (512, 512), (1024, 512), (1536, 512)])
                for ci, (tcs, tch) in enumerate(chunks):
                    if tcs == 0:
                        xca, xcb = x_pre
                    else:
                        xca, xcb = load_x(tcs, tch)

                    def xk(ke):
                        return (xca if ke < KE // 2
                                else xcb)[:, ke % (KE // 2), :]
                    tcol = slice(tcs, tcs + tch)
                    # q sections for both pairs first, then k sections
                    for sec, pi in [(0, 0), (0, 1), (256, 0), (256, 1)]:
                        dst = pair_qk[pi][0 if sec == 0 else 1]
                        ps = psA.tile([128, tch], F32, tag="psA", name="ps")
                        wcol = sec + 128 * pi
                        for ke in range(KE):
                            mm(ps, wg[:, ke, wcol:wcol + 128],
                               xk(ke), start=(ke == 0),
                               stop=(ke == KE - 1), skip_group_check=True)
                        # rope: dst = ps*cos + swap(ps*ssp) -- swap on DVE
                        bt0 = tmp.tile([128, tch], F32, tag="bt0", bufs=1)
                        nc.vector.tensor_mul(bt0, ps, c_ssp[:, tcol])
                        bt = tmp.tile([128, tch], F32, tag="bt", bufs=1)
                        nc.vector.stream_shuffle(bt, bt0, SWAP_MASK)
                        nc.vector.tensor_mul(dst[:, tcol], ps, c_cos[:, tcol])
                        nc.vector.tensor_add(
                            dst[:, tcol], dst[:, tcol].bitcast(F32), bt)
                    # v for both pairs
                    for ti in range(tch // 128):
                        tt = (tcs + ti * 128) // 128
                        psv = psA.tile([128, 256], F32, tag="psA", name="psv")
                        for ke in range(KE):
                            mm(psv, xk(ke)[:, 128 * ti:128 * ti + 128],
                               wg[:, ke, 512:768], start=(ke == 0),
                               stop=(ke == KE - 1), skip_group_check=True)
                        for pi in range(2):
                            nc.scalar.copy(
                                pair_v[pi][:, tt, 0:64],
                                psv[:, 128 * pi:128 * pi + 64])
                            nc.scalar.copy(
                                pair_v[pi][:, tt, 65:129],
                                psv[:, 128 * pi + 64:128 * pi + 128])

                    if g == 0 and ci == len(chunks) - 1:
                        # prefetch group-1 weights + first x chunk + w_proj;
                        # issued after the last group-0 qkv matmul is queued
                        # so the WAR wait clears before group-0's final
                        # attention chunk, which covers the transfer time
                        wg1, j0g1 = load_wg(1)
                        x_pre = load_x(0, 512)
                        nc.sync.dma_start(out=wg1[:, :, 256:512],
                                          in_=wk_r[:, :, j0g1:j0g1 + 256])
                        nc.sync.dma_start(out=wg1[:, :, 512:768],
                                          in_=wv_r[:, :, j0g1:j0g1 + 256])
                        nc.sync.dma_start(out=c_wp, in_=wp_r)

                    end = tcs + tch
                    if end % 512 == 0:
                        qq = end // 512 - 1
                        for pi in range(2):
                            attention_chunk(2 * g + pi, qq, *pair_qk[pi],
                                            pair_v[pi], pair_y[pi])
                if g == 0:
                    wg = wg1
                else:
                    _proj(nc, mm, psA, outp, c_wp, y_tiles, outT, 3)

    nc.compile()
    _NC_CACHE["nc"] = nc
    return nc


def _proj(nc, mm, psA, outp, c_wp, y_tiles, outT, qq):
    qb = 512 * qq
    for mo in range(E // 128):
        po = psA.tile([128, 512], F32, tag="psA", name="po")
        for kp in range(NPAIR):
            mm(po, c_wp[:, kp, 128 * mo:128 * mo + 128],
               y_tiles[kp][:, qb:qb + 512],
               start=(kp == 0), stop=(kp == NPAIR - 1),
               skip_group_check=True)
        ost = outp.tile([128, 512], F32, tag="ost")
        nc.vector.tensor_copy(ost, po)
        nc.sync.dma_start(out=outT[128 * mo:128 * mo + 128, qb:qb + 512],
                          in_=ost)


def _host_tables():
    inv_freq = 1.0 / (10000.0 ** (np.arange(0, D, 2, dtype=np.float32) / D))
    t = np.arange(T, dtype=np.float32)
    freqs = np.outer(t, inv_freq)                     # [T, 32]
    emb = np.concatenate([freqs, freqs], -1)          # [T, 64]
    cos_t = np.cos(emb).T.astype(np.float32)          # [64, T]
    sin_t = np.sin(emb).T.astype(np.float32)
    # rope(x)[d] = x[d]*cos[d] + x[d^1]*ssin[d],
    #   ssin[2i] = -sin[2i], ssin[2i+1] = +sin[2i+1]
    # device computes swap(x * ssp) with swap[d] = d^1, so ssp[d] = ssin[d^1]
    ssp = np.empty_like(sin_t)
    ssp[0::2] = sin_t[1::2]       # even d: +sin(emb[d+1])
    ssp[1::2] = -sin_t[0::2]      # odd d:  -sin(emb[d-1])
    cos2 = np.concatenate([cos_t, cos_t], 0)          # [128, T]
    ssp2 = np.concatenate([ssp, ssp], 0)
    r = np.arange(128)
    maskd = (r[:, None] <= r[None, :]).astype(np.float32)
    return cos2, ssp2, maskd


def kernel(x, w_attn, w_proj):
    x = np.asarray(x, dtype=np.float32)
    w_attn = np.asarray(w_attn, dtype=np.float32)
    w_proj = np.asarray(w_proj, dtype=np.float32)
    cos2, ssp2, maskd = _host_tables()

    nc = _build_program()
    in_maps = []
    for c in range(NCORES):
        b, g = c // 2, c % 2
        j0 = g * F
        in_maps.append({
            "xT": np.ascontiguousarray(x[b].T),
            "wqT": np.ascontiguousarray(w_attn[j0:j0 + F].T),
            "wkT": np.ascontiguousarray(w_attn[E + j0:E + j0 + F].T),
            "wvT": np.ascontiguousarray(w_attn[2 * E + j0:2 * E + j0 + F].T),
            "wpT": np.ascontiguousarray(w_proj[:, j0:j0 + F].T),
            "cos2": cos2, "ssp2": ssp2, "maskd": maskd,
            "ones16": np.ones((128, 16), dtype=np.float32),
        })
    res = run_bass_kernel_spmd(nc, in_maps, core_ids=list(range(NCORES)))
    out = np.empty((B, T, E), dtype=np.float32)
    for b in range(B):
        acc = res.results[2 * b]["outT"] + res.results[2 * b + 1]["outT"]
        out[b] = acc.T
    return out



# revision 10
# speedup vs baseline: 1.1031x; 1.1031x over previous
"""Causal self-attention (B=4, T=2048, E=1024, H=16, D=64) on 8 trn2 cores.

Sharding: core c -> (batch b = c//2, head-group g = c%2 of 8 heads).
Each core computes qkv projection + RoPE + causal attention + its partial
output projection for its (batch, head-group); host sums the two
head-group partials per batch and transposes back.

Device data layout is feature-major ("T" suffix = [features, tokens]):
scores are computed k-major (S.T blocks [tk=128, tq]) so causal masking
skips ~half the matmuls, and softmax normalization comes from an extra
ones-column in the v operand of the PV matmul (the denominator lands in
one PSUM partition row at zero extra matmul cost).

All matmuls run in float32r (full PE rate for N>=256); diagonal blocks
that would fall to N=128 (4 cyc/row in fp32r) are widened to N=256 —
the extra score columns land in unread PSUM, and the extra PV columns
read zeros memset into pt.

v5 scheduling: the program is emitted as software-pipelined "rounds".
Round(qq) interleaves, at 3-kt granularity, the attention of q-chunk qq
(scores/exp/mask/PV on PE+Act+Pool) with the qkv+RoPE sections of the
NEXT chunk (PE+DVE+Pool) and the output projection of a completed
q-chunk, so the PE never drains its pipeline while the activation
engine catches up on exp. Elementwise work is spread across all three
non-PE engines:
  DVE:  rope muls (PSUM drain) + swap shuffle, drain shuffle/recip,
        hl1 drain mul (cross-partition write), half the psum copies
  Pool: rope adds, causal mask muls, hl0 drain mul, pt memsets
  Act:  exp, psy drain copies, half the psum copies
The drain broadcasts the denominator row with ONE 64-row stream_shuffle
(denominator duplicated to partition 96 first), and y_sb pool buffers
are memset once at startup so the shuffle never reads uninitialized
SBUF (keeps CoreSim clean).
"""
import sys

sys.path.insert(0, "/opt/trn_rl_repo")

from contextlib import ExitStack

import numpy as np

import concourse.bass as bass
import concourse.bacc as bacc
import concourse.tile as tile
from concourse import mybir
from concourse.bass_utils import run_bass_kernel_spmd

B, T, E, H, D = 4, 2048, 1024, 16, 64
NCORES = 8
HG = H // 2          # heads per shard (8)
F = HG * D           # features per shard (512)
NPAIR = F // 128     # head pairs per shard (4)
NGRP = NPAIR // 2    # pair groups (2)
KE = E // 128        # contraction tiles over E (8)
NKT = T // 128       # k tiles (16)
F32 = mybir.dt.float32
F32R = mybir.dt.float32r
EXP = mybir.ActivationFunctionType.Exp
SWAP_MASK = [i ^ 1 for i in range(32)]   # rope pair swap within banks
BCAST_MASK = [0] * 32                     # all lanes take lane 0

_NC_CACHE = {}


def _build_program():
    if "nc" in _NC_CACHE:
        return _NC_CACHE["nc"]
    nc = bacc.Bacc("TRN2", target_bir_lowering=False, debug=False,
                   num_devices=NCORES)
    mm = nc.tensor.matmul
    xT = nc.dram_tensor("xT", [E, T], F32R, kind="ExternalInput").ap()
    wqT = nc.dram_tensor("wqT", [E, F], F32R, kind="ExternalInput").ap()
    wkT = nc.dram_tensor("wkT", [E, F], F32R, kind="ExternalInput").ap()
    wvT = nc.dram_tensor("wvT", [E, F], F32R, kind="ExternalInput").ap()
    wpT = nc.dram_tensor("wpT", [F, E], F32R, kind="ExternalInput").ap()
    cos2 = nc.dram_tensor("cos2", [128, T], F32, kind="ExternalInput").ap()
    ssp2 = nc.dram_tensor("ssp2", [128, T], F32, kind="ExternalInput").ap()
    maskd = nc.dram_tensor("maskd", [128, 128], F32, kind="ExternalInput").ap()
    ones16 = nc.dram_tensor("ones16", [128, NKT], F32R,
                            kind="ExternalInput").ap()
    outT = nc.dram_tensor("outT", [E, T], F32, kind="ExternalOutput").ap()

    xT_r = xT.rearrange("(ke p) t -> p ke t", p=128)
    wq_r = wqT.rearrange("(ke p) j -> p ke j", p=128)
    wk_r = wkT.rearrange("(ke p) j -> p ke j", p=128)
    wv_r = wvT.rearrange("(ke p) j -> p ke j", p=128)
    wp_r = wpT.rearrange("(kf p) o -> p kf o", p=128)

    with tile.TileContext(nc) as tc:
        with ExitStack() as ctx:
            const = ctx.enter_context(tc.tile_pool(name="const", bufs=1))
            wgp = ctx.enter_context(tc.tile_pool(name="wgp", bufs=1))
            xp = ctx.enter_context(tc.tile_pool(name="xp", bufs=2))
            qkp = ctx.enter_context(tc.tile_pool(name="qkp", bufs=2))
            vp = ctx.enter_context(tc.tile_pool(name="vp", bufs=3))
            yp = ctx.enter_context(tc.tile_pool(name="yp", bufs=4))
            pp = ctx.enter_context(tc.tile_pool(name="pp", bufs=4))
            tmp = ctx.enter_context(tc.tile_pool(name="tmp", bufs=2))
            ysbp = ctx.enter_context(tc.tile_pool(name="ysbp", bufs=3))
            bcp = ctx.enter_context(tc.tile_pool(name="bcp", bufs=2))
            outp = ctx.enter_context(tc.tile_pool(name="outp", bufs=2))
            psA = ctx.enter_context(
                tc.tile_pool(name="psA", bufs=2, space="PSUM"))
            psS = ctx.enter_context(
                tc.tile_pool(name="psS", bufs=2, space="PSUM"))
            psY = ctx.enter_context(
                tc.tile_pool(name="psY", bufs=2, space="PSUM"))

            c_cos = const.tile([128, T], F32, tag="cos")
            c_ssp = const.tile([128, T], F32, tag="ssp")
            c_mask = const.tile([128, 128], F32, tag="mask")
            c_wp = const.tile([128, NPAIR, E], F32R, tag="wp")

            # y_sb drain buffers: persistent tiles, reused round-robin by
            # the drains (subtile WAR deps order the reuse). Rows 65:96 and
            # 97:128 are read by the 64-row broadcast shuffle but never
            # written per-drain — memset them once here.
            ysb_tiles = []
            for i in range(3):
                t = ysbp.tile([128, 512], F32, tag="ysb", name=f"ysb{i}")
                nc.vector.memset(t[64:96, :], 0)
                nc.vector.memset(t[96:128, :], 0)
                ysb_tiles.append(t)
            ysb_ctr = [0]

            def next_ysb():
                t = ysb_tiles[ysb_ctr[0] % 3]
                ysb_ctr[0] += 1
                return t

            pairs = [None] * NPAIR   # (qT, kT, v3, yT) per global pair

            def alloc_pair(p, with_ones=True):
                qT = qkp.tile([128, T], F32R, tag="qT", name=f"qT{p}")
                kT = qkp.tile([128, T], F32R, tag="kT", name=f"kT{p}")
                v3 = vp.tile([128, NKT, 130], F32R, tag="v3", name=f"v3{p}")
                yT = yp.tile([128, T], F32R, tag="yT", name=f"yT{p}")
                pairs[p] = (qT, kT, v3, yT)
                if with_ones:
                    emit_ones(p)

            def emit_ones(p):
                v3 = pairs[p][2]
                nc.sync.dma_start(out=v3[:, :, 64], in_=ones16)
                nc.sync.dma_start(out=v3[:, :, 129], in_=ones16)

            def load_x(tcs, tch):
                xca = xp.tile([128, KE // 2, tch], F32R, tag="xc",
                              name=f"xca{tcs}")
                xcb = xp.tile([128, KE // 2, tch], F32R, tag="xc",
                              name=f"xcb{tcs}")
                nc.sync.dma_start(out=xca,
                                  in_=xT_r[:, 0:KE // 2, tcs:tcs + tch])
                nc.sync.dma_start(out=xcb,
                                  in_=xT_r[:, KE // 2:KE, tcs:tcs + tch])
                return xca, xcb

            def make_xk(xt):
                xca, xcb = xt
                return lambda ke: (xca if ke < KE // 2
                                   else xcb)[:, ke % (KE // 2), :]

            # ---------------- emit helpers ----------------

            def emit_section(wg, xk, tcs, tch, sec, pi, dst):
                """One 128-feature q or k section: 8 matmuls + rope."""
                tcol = slice(tcs, tcs + tch)
                ps = psA.tile([128, tch], F32, tag="psA", name="ps")
                wcol = sec + 128 * pi
                for ke in range(KE):
                    mm(ps, wg[:, ke, wcol:wcol + 128], xk(ke),
                       start=(ke == 0), stop=(ke == KE - 1),
                       skip_group_check=True)
                # rope: dst = ps*cos + swap(ps*ssp); psA freed by the 2 muls
                bt0 = tmp.tile([128, tch], F32, tag="bt0", bufs=2)
                nc.vector.tensor_mul(bt0, ps, c_ssp[:, tcol])
                ct = tmp.tile([128, tch], F32, tag="ct", bufs=2)
                nc.vector.tensor_mul(ct, ps, c_cos[:, tcol])
                bt = tmp.tile([128, tch], F32, tag="bt", bufs=2)
                nc.vector.stream_shuffle(bt, bt0, SWAP_MASK)
                nc.gpsimd.tensor_add(dst[:, tcol], ct, bt)

            def emit_vtile(wg, xk, tcs, ti, g):
                """v for both pairs of group g at token tile ti of chunk."""
                tt = (tcs + 128 * ti) // 128
                psv = psA.tile([128, 256], F32, tag="psA", name="psv")
                for ke in range(KE):
                    mm(psv, xk(ke)[:, 128 * ti:128 * ti + 128],
                       wg[:, ke, 512:768], start=(ke == 0),
                       stop=(ke == KE - 1), skip_group_check=True)
                for pi in range(2):
                    v3 = pairs[2 * g + pi][2]
                    cp = (nc.vector.tensor_copy if ti % 2 == 0
                          else nc.scalar.copy)
                    cp(v3[:, tt, 0:64], psv[:, 128 * pi:128 * pi + 64])
                    cp(v3[:, tt, 65:129],
                       psv[:, 128 * pi + 64:128 * pi + 128])

            def attn_gen(p, qq, blk=4):
                """Attention for pair p, q-chunk qq; yields every blk kts."""
                qT, kT, v3, yT = pairs[p]
                qb = 512 * qq
                last = 4 * qq + 3
                psy0 = psY.tile([128, 512], F32, tag="psY", name="psy0")
                psy1 = psY.tile([128, 512], F32, tag="psY", name="psy1")
                psy = (psy0, psy1)

                def pv(item):
                    kt, pt, lo = item
                    for hl in range(2):
                        mm(psy[hl][0:65, lo:512],
                           v3[:, kt, 65 * hl:65 * hl + 65],
                           pt[:, 512 * hl + lo:512 * hl + 512],
                           start=(kt == 0), stop=(kt == last),
                           skip_group_check=True)

                prev = None
                for kt in range(last + 1):
                    col_lo = max(qb, 128 * kt) - qb
                    mm_lo = 256 if col_lo == 384 else col_lo
                    pS = psS.tile([128, 1024], F32, tag="psS")
                    for hl in range(2):
                        hr = 64 * hl
                        mm(pS[:, 512 * hl + mm_lo:512 * hl + 512],
                           kT[hr:hr + 64, 128 * kt:128 * kt + 128],
                           qT[hr:hr + 64, qb + mm_lo:qb + 512],
                           start=True, stop=True, skip_group_check=True)
                    pt = pp.tile([128, 1024], F32R, tag="pt")
                    pS2 = pS.rearrange("p (h c) -> p h c", h=2)
                    pt2 = pt.rearrange("p (h c) -> p h c", h=2)
                    if mm_lo != col_lo:
                        # widened PV reads [256:384] — must be zeros
                        nc.gpsimd.memset(pt2[:, :, 256:384].bitcast(F32), 0)
                    nc.scalar.activation(
                        pt2[:, :, col_lo:512], pS2[:, :, col_lo:512],
                        EXP, scale=0.125)
                    if 128 * kt >= qb:  # diagonal block, both heads
                        # split across DVE and Pool so the two mask muls
                        # run in parallel (PV waits on both)
                        o = col_lo
                        nc.vector.tensor_mul(
                            pt[:, o:o + 128],
                            pt[:, o:o + 128].bitcast(F32), c_mask)
                        o = 512 + col_lo
                        nc.gpsimd.tensor_mul(
                            pt[:, o:o + 128],
                            pt[:, o:o + 128].bitcast(F32), c_mask)
                    if prev is not None:
                        pv(prev)
                    prev = (kt, pt, mm_lo)
                    if (kt + 1) % blk == 0 and kt != last:
                        yield
                pv(prev)
                # ---- softmax drain ----
                # both psum copies first (scalar || DVE) so the psY banks
                # free promptly for the next attention chunk
                ys = (next_ysb(), next_ysb())
                nc.scalar.copy(ys[0][0:65, :], psy[0][0:65, :])
                nc.vector.tensor_copy(ys[1][0:65, :], psy[1][0:65, :])
                col = slice(qb, qb + 512)
                for hl in range(2):
                    y_sb = ys[hl]
                    nc.vector.tensor_copy(y_sb[96:97, :], y_sb[64:65, :])
                    bcr = bcp.tile([64, 512], F32, tag="bcr")
                    nc.vector.stream_shuffle(bcr, y_sb[64:128, :],
                                             BCAST_MASK)
                    bc = bcp.tile([64, 512], F32, tag="bc")
                    nc.vector.reciprocal_approx_fast(bc, bcr)
                    if hl == 0:
                        nc.gpsimd.tensor_mul(yT[0:64, col],
                                             y_sb[0:64, :], bc)
                    else:
                        nc.vector.tensor_mul(yT[64:128, col],
                                             y_sb[0:64, :], bc)

            def emit_proj(qq):
                qb = 512 * qq
                for mo in range(E // 128):
                    po = psA.tile([128, 512], F32, tag="psA", name="po")
                    for kp in range(NPAIR):
                        mm(po, c_wp[:, kp, 128 * mo:128 * mo + 128],
                           pairs[kp][3][:, qb:qb + 512],
                           start=(kp == 0), stop=(kp == NPAIR - 1),
                           skip_group_check=True)
                    ost = outp.tile([128, 512], F32, tag="ost")
                    (nc.vector.tensor_copy if mo % 2 == 0
                     else nc.scalar.copy)(ost, po)
                    nc.sync.dma_start(
                        out=outT[128 * mo:128 * mo + 128, qb:qb + 512],
                        in_=ost)

            def drive(gen, fillers):
                fi = 0
                for _ in gen:
                    if fi < len(fillers):
                        fillers[fi]()
                        fi += 1
                while fi < len(fillers):
                    fillers[fi]()
                    fi += 1

            def chunk_fillers(wg, xt, tcs, g):
                """Section/v fillers for a 512-col chunk of group g."""
                xk = make_xk(xt)
                qk = [pairs[2 * g], pairs[2 * g + 1]]
                return [
                    lambda: emit_section(wg, xk, tcs, 512, 0, 0, qk[0][0]),
                    lambda: emit_section(wg, xk, tcs, 512, 256, 0, qk[0][1]),
                    lambda: (emit_vtile(wg, xk, tcs, 0, g),
                             emit_vtile(wg, xk, tcs, 1, g)),
                    lambda: (emit_vtile(wg, xk, tcs, 2, g),
                             emit_vtile(wg, xk, tcs, 3, g)),
                    lambda: emit_section(wg, xk, tcs, 512, 0, 1, qk[1][0]),
                    lambda: emit_section(wg, xk, tcs, 512, 256, 1, qk[1][1]),
                ]

            # ---------------- schedule ----------------

            alloc_pair(0)
            alloc_pair(1)

            # startup DMAs: first-needed bytes first, fine-grained
            wg0 = wgp.tile([128, KE, 768], F32R, tag="wg", name="wg0")
            nc.sync.dma_start(out=wg0[:, 0:4, 0:256], in_=wq_r[:, 0:4, 0:256])
            x_a = load_x(0, 256)
            nc.sync.dma_start(out=wg0[:, 4:8, 0:256], in_=wq_r[:, 4:8, 0:256])
            nc.sync.dma_start(out=c_cos[:, 0:256], in_=cos2[:, 0:256])
            nc.sync.dma_start(out=c_ssp[:, 0:256], in_=ssp2[:, 0:256])
            nc.sync.dma_start(out=wg0[:, 0:4, 256:512],
                              in_=wk_r[:, 0:4, 0:256])
            nc.sync.dma_start(out=wg0[:, 4:8, 256:512],
                              in_=wk_r[:, 4:8, 0:256])
            x_b = load_x(256, 256)
            nc.sync.dma_start(out=c_cos[:, 256:512], in_=cos2[:, 256:512])
            nc.sync.dma_start(out=c_ssp[:, 256:512], in_=ssp2[:, 256:512])
            nc.sync.dma_start(out=wg0[:, :, 512:768], in_=wv_r[:, :, 0:256])
            nc.sync.dma_start(out=c_mask, in_=maskd)
            for cc in range(1, 4):
                nc.sync.dma_start(out=c_cos[:, 512 * cc:512 * cc + 512],
                                  in_=cos2[:, 512 * cc:512 * cc + 512])
                nc.sync.dma_start(out=c_ssp[:, 512 * cc:512 * cc + 512],
                                  in_=ssp2[:, 512 * cc:512 * cc + 512])

            # startup chunks (group 0, 256+256): q sections first (wq is
            # in flight first), then k, then v
            for tcs, xt in ((0, x_a), (256, x_b)):
                xk = make_xk(xt)
                emit_section(wg0, xk, tcs, 256, 0, 0, pairs[0][0])
                emit_section(wg0, xk, tcs, 256, 0, 1, pairs[1][0])
                emit_section(wg0, xk, tcs, 256, 256, 0, pairs[0][1])
                emit_section(wg0, xk, tcs, 256, 256, 1, pairs[1][1])
                emit_vtile(wg0, xk, tcs, 0, 0)
                emit_vtile(wg0, xk, tcs, 1, 0)

            # rounds 0-2: attention(g0, qq) x qkv(g0, chunk qq+1)
            for qq in range(3):
                tcs = 512 * (qq + 1)
                xt = load_x(tcs, 512)
                f = chunk_fillers(wg0, xt, tcs, 0)
                drive(attn_gen(0, qq), f[:3])
                drive(attn_gen(1, qq), f[3:])

            # group-1 prefetch (WAR on wg0 clears: all g0 qkv mms emitted)
            wg1 = wgp.tile([128, KE, 768], F32R, tag="wg", name="wg1")
            nc.sync.dma_start(out=wg1[:, :, 0:256], in_=wq_r[:, :, 256:512])
            nc.sync.dma_start(out=wg1[:, :, 512:768],
                              in_=wv_r[:, :, 256:512])
            x_g1 = load_x(0, 512)
            nc.sync.dma_start(out=wg1[:, :, 256:512],
                              in_=wk_r[:, :, 256:512])
            nc.sync.dma_start(out=c_wp, in_=wp_r)
            alloc_pair(2)   # 3rd buffer in vp — no WAR with live g0 reads

            # round 3: attention(g0, 3); g1-chunk0 qkv must stay out of
            # gen0's zone (its writes WAR-target buffers gen0 still reads)
            xk1 = make_xk(x_g1)
            drive(attn_gen(0, 3), [])
            drive(attn_gen(1, 3), [
                lambda: emit_section(wg1, xk1, 0, 512, 0, 0, pairs[2][0]),
                lambda: emit_section(wg1, xk1, 0, 512, 256, 0, pairs[2][1]),
            ])
            alloc_pair(3)
            emit_section(wg1, xk1, 0, 512, 0, 1, pairs[3][0])
            emit_section(wg1, xk1, 0, 512, 256, 1, pairs[3][1])
            for ti in range(4):
                emit_vtile(wg1, xk1, 0, ti, 1)

            # rounds 4-7: attention(g1, qq) x qkv(g1, chunk qq+1) + proj
            for qq in range(4):
                if qq < 3:
                    tcs = 512 * (qq + 1)
                    xt = load_x(tcs, 512)
                    f = chunk_fillers(wg1, xt, tcs, 1)
                    drive(attn_gen(2, qq), f[:3])
                    drive(attn_gen(3, qq), f[3:])
                    if qq < 2:
                        emit_proj(qq)
                else:
                    drive(attn_gen(2, 3), [])
                    drive(attn_gen(3, 3), [lambda: emit_proj(2)])
                    emit_proj(3)

    nc.compile()
    _NC_CACHE["nc"] = nc
    return nc


def _host_tables():
    inv_freq = 1.0 / (10000.0 ** (np.arange(0, D, 2, dtype=np.float32) / D))
    t = np.arange(T, dtype=np.float32)
    freqs = np.outer(t, inv_freq)                     # [T, 32]
    emb = np.concatenate([freqs, freqs], -1)          # [T, 64]
    cos_t = np.cos(emb).T.astype(np.float32)          # [64, T]
    sin_t = np.sin(emb).T.astype(np.float32)
    # rope(x)[d] = x[d]*cos[d] + x[d^1]*ssin[d],
    #   ssin[2i] = -sin[2i], ssin[2i+1] = +sin[2i+1]
    # device computes swap(x * ssp) with swap[d] = d^1, so ssp[d] = ssin[d^1]
    ssp = np.empty_like(sin_t)
    ssp[0::2] = sin_t[1::2]       # even d: +sin(emb[d+1])
    ssp[1::2] = -sin_t[0::2]      # odd d:  -sin(emb[d-1])
    cos2 = np.concatenate([cos_t, cos_t], 0)          # [128, T]
    ssp2 = np.concatenate([ssp, ssp], 0)
    r = np.arange(128)
    maskd = (r[:, None] <= r[None, :]).astype(np.float32)
    return cos2, ssp2, maskd


def _core_inputs(x, w_attn, w_proj, c):
    cos2, ssp2, maskd = _host_tables()
    b, g = c // 2, c % 2
    j0 = g * F
    return {
        "xT": np.ascontiguousarray(x[b].T),
        "wqT": np.ascontiguousarray(w_attn[j0:j0 + F].T),
        "wkT": np.ascontiguousarray(w_attn[E + j0:E + j0 + F].T),
        "wvT": np.ascontiguousarray(w_attn[2 * E + j0:2 * E + j0 + F].T),
        "wpT": np.ascontiguousarray(w_proj[:, j0:j0 + F].T),
        "cos2": cos2, "ssp2": ssp2, "maskd": maskd,
        "ones16": np.ones((128, 16), dtype=np.float32),
    }


def kernel(x, w_attn, w_proj):
    x = np.asarray(x, dtype=np.float32)
    w_attn = np.asarray(w_attn, dtype=np.float32)
    w_proj = np.asarray(w_proj, dtype=np.float32)

    nc = _build_program()
    in_maps = [_core_inputs(x, w_attn, w_proj, c) for c in range(NCORES)]
    res = run_bass_kernel_spmd(nc, in_maps, core_ids=list(range(NCORES)))
    out = np.empty((B, T, E), dtype=np.float32)
    for b in range(B):
        acc = res.results[2 * b]["outT"] + res.results[2 * b + 1]["outT"]
        out[b] = acc.T
    return out


# revision 25
# speedup vs baseline: 1.1316x; 1.0259x over previous
"""Causal self-attention (B=4, T=2048, E=1024, H=16, D=64) on 8 trn2 cores.

Sharding: core c -> (batch b = c//2, head-group g = c%2 of 8 heads).
Each core computes qkv projection + RoPE + causal attention + its partial
output projection for its (batch, head-group); host sums the two
head-group partials per batch and transposes back.

Device data layout is feature-major ("T" suffix = [features, tokens]):
scores are computed k-major (S.T blocks [tk=128, tq]) so causal masking
skips ~half the matmuls, and softmax normalization comes from an extra
ones-column in the v operand of the PV matmul (the denominator lands in
one PSUM partition row at zero extra matmul cost).

All matmuls run in float32r (full PE rate for N>=256); diagonal blocks
that would fall to N=128 (4 cyc/row in fp32r) are widened to N=256 —
the extra score columns land in unread PSUM, and the extra PV columns
read zeros memset into pt.

v5 scheduling: the program is emitted as software-pipelined "rounds".
Round(qq) interleaves, at 3-kt granularity, the attention of q-chunk qq
(scores/exp/mask/PV on PE+Act+Pool) with the qkv+RoPE sections of the
NEXT chunk (PE+DVE+Pool) and the output projection of a completed
q-chunk, so the PE never drains its pipeline while the activation
engine catches up on exp. Elementwise work is spread across all three
non-PE engines:
  DVE:  rope muls (PSUM drain) + swap shuffle, drain shuffle/recip,
        hl1 drain mul (cross-partition write), half the psum copies
  Pool: rope adds, causal mask muls, hl0 drain mul, pt memsets
  Act:  exp, psy drain copies, half the psum copies
The drain broadcasts the denominator row with ONE 64-row stream_shuffle
(denominator duplicated to partition 96 first), and y_sb pool buffers
are memset once at startup so the shuffle never reads uninitialized
SBUF (keeps CoreSim clean).
"""
import sys

sys.path.insert(0, "/opt/trn_rl_repo")

from contextlib import ExitStack

import numpy as np

import concourse.bass as bass
import concourse.bacc as bacc
import concourse.tile as tile
from concourse import mybir
from concourse.bass_utils import run_bass_kernel_spmd

B, T, E, H, D = 4, 2048, 1024, 16, 64
NCORES = 8
HG = H // 2          # heads per shard (8)
F = HG * D           # features per shard (512)
NPAIR = F // 128     # head pairs per shard (4)
NGRP = NPAIR // 2    # pair groups (2)
KE = E // 128        # contraction tiles over E (8)
NKT = T // 128       # k tiles (16)
F32 = mybir.dt.float32
F32R = mybir.dt.float32r
EXP = mybir.ActivationFunctionType.Exp
SWAP_MASK = [i ^ 1 for i in range(32)]   # rope pair swap within banks
BCAST_MASK = [0] * 32                     # all lanes take lane 0

_NC_CACHE = {}


def _build_program():
    if "nc" in _NC_CACHE:
        return _NC_CACHE["nc"]
    nc = bacc.Bacc("TRN2", target_bir_lowering=False, debug=False,
                   num_devices=NCORES)
    mm = nc.tensor.matmul
    xT = nc.dram_tensor("xT", [E, T], F32R, kind="ExternalInput").ap()
    wqT = nc.dram_tensor("wqT", [E, F], F32R, kind="ExternalInput").ap()
    wkT = nc.dram_tensor("wkT", [E, F], F32R, kind="ExternalInput").ap()
    wvT = nc.dram_tensor("wvT", [E, F], F32R, kind="ExternalInput").ap()
    wpT = nc.dram_tensor("wpT", [F, E], F32R, kind="ExternalInput").ap()
    cos2 = nc.dram_tensor("cos2", [128, T], F32, kind="ExternalInput").ap()
    ssp2 = nc.dram_tensor("ssp2", [128, T], F32, kind="ExternalInput").ap()
    maskd = nc.dram_tensor("maskd", [128, 128], F32, kind="ExternalInput").ap()
    ones16 = nc.dram_tensor("ones16", [128, NKT], F32R,
                            kind="ExternalInput").ap()
    outT = nc.dram_tensor("outT", [E, T], F32, kind="ExternalOutput").ap()

    xT_r = xT.rearrange("(ke p) t -> p ke t", p=128)
    wq_r = wqT.rearrange("(ke p) j -> p ke j", p=128)
    wk_r = wkT.rearrange("(ke p) j -> p ke j", p=128)
    wv_r = wvT.rearrange("(ke p) j -> p ke j", p=128)
    wp_r = wpT.rearrange("(kf p) o -> p kf o", p=128)

    with tile.TileContext(nc) as tc:
        with ExitStack() as ctx:
            const = ctx.enter_context(tc.tile_pool(name="const", bufs=1))
            wgp = ctx.enter_context(tc.tile_pool(name="wgp", bufs=1))
            xp = ctx.enter_context(tc.tile_pool(name="xp", bufs=2))
            qkp = ctx.enter_context(tc.tile_pool(name="qkp", bufs=2))
            vp = ctx.enter_context(tc.tile_pool(name="vp", bufs=3))
            yp = ctx.enter_context(tc.tile_pool(name="yp", bufs=4))
            pp = ctx.enter_context(tc.tile_pool(name="pp", bufs=4))
            tmp = ctx.enter_context(tc.tile_pool(name="tmp", bufs=2))
            ysbp = ctx.enter_context(tc.tile_pool(name="ysbp", bufs=2))
            bcp = ctx.enter_context(tc.tile_pool(name="bcp", bufs=2))
            outp = ctx.enter_context(tc.tile_pool(name="outp", bufs=3))
            psA = ctx.enter_context(
                tc.tile_pool(name="psA", bufs=2, space="PSUM"))
            psS = ctx.enter_context(
                tc.tile_pool(name="psS", bufs=2, space="PSUM"))
            psY = ctx.enter_context(
                tc.tile_pool(name="psY", bufs=2, space="PSUM"))

            c_cos = const.tile([128, T], F32, tag="cos")
            c_ssp = const.tile([128, T], F32, tag="ssp")
            c_mask = const.tile([128, 128], F32, tag="mask")
            c_wp = const.tile([128, NPAIR, E], F32R, tag="wp")

            # y_sb drain buffers: persistent tiles, reused round-robin by
            # the drains (subtile WAR deps order the reuse). Rows 65:96 and
            # 97:128 are read by the 64-row broadcast shuffle but never
            # written per-drain — memset them once here.
            ysb_tiles = []
            for i in range(2):
                t = ysbp.tile([128, 512], F32, tag="ysb", name=f"ysb{i}")
                nc.vector.memset(t[64:96, :], 0)
                nc.vector.memset(t[96:128, :], 0)
                ysb_tiles.append(t)
            ysb_ctr = [0]

            def next_ysb():
                t = ysb_tiles[ysb_ctr[0] % 2]
                ysb_ctr[0] += 1
                return t

            pairs = [None] * NPAIR   # (qT, kT, v3, yT) per global pair

            def alloc_pair(p, with_ones=True):
                qT = qkp.tile([128, T], F32R, tag="qT", name=f"qT{p}")
                kT = qkp.tile([128, T], F32R, tag="kT", name=f"kT{p}")
                v3 = vp.tile([128, NKT, 130], F32R, tag="v3", name=f"v3{p}")
                yT = yp.tile([128, T], F32R, tag="yT", name=f"yT{p}")
                pairs[p] = (qT, kT, v3, yT)
                if with_ones:
                    emit_ones(p)

            def emit_ones(p):
                v3 = pairs[p][2]
                nc.sync.dma_start(out=v3[:, :, 64], in_=ones16)
                nc.sync.dma_start(out=v3[:, :, 129], in_=ones16)

            def load_x(tcs, tch):
                xca = xp.tile([128, KE // 2, tch], F32R, tag="xc",
                              name=f"xca{tcs}")
                xcb = xp.tile([128, KE // 2, tch], F32R, tag="xc",
                              name=f"xcb{tcs}")
                nc.sync.dma_start(out=xca,
                                  in_=xT_r[:, 0:KE // 2, tcs:tcs + tch])
                nc.sync.dma_start(out=xcb,
                                  in_=xT_r[:, KE // 2:KE, tcs:tcs + tch])
                return xca, xcb

            def make_xk(xt):
                xca, xcb = xt
                return lambda ke: (xca if ke < KE // 2
                                   else xcb)[:, ke % (KE // 2), :]

            # ---------------- emit helpers ----------------

            def emit_section(wg, xk, tcs, tch, sec, pi, dst):
                """One 128-feature q or k section: 8 matmuls + rope."""
                tcol = slice(tcs, tcs + tch)
                ps = psA.tile([128, tch], F32, tag="psA", name="ps")
                wcol = sec + 128 * pi
                for ke in range(KE):
                    mm(ps, wg[:, ke, wcol:wcol + 128], xk(ke),
                       start=(ke == 0), stop=(ke == KE - 1),
                       skip_group_check=True)
                # rope: dst = ps*cos + swap(ps*ssp); psA freed by the 2 muls
                bt0 = tmp.tile([128, tch], F32, tag="bt0", bufs=1)
                nc.vector.tensor_mul(bt0, ps, c_ssp[:, tcol])
                ct = tmp.tile([128, tch], F32, tag="ct", bufs=2)
                nc.vector.tensor_mul(ct, ps, c_cos[:, tcol])
                bt = tmp.tile([128, tch], F32, tag="bt", bufs=2)
                nc.vector.stream_shuffle(bt, bt0, SWAP_MASK)
                nc.gpsimd.tensor_add(dst[:, tcol], ct, bt)

            def emit_vtile(wg, xk, tcs, ti, g):
                """v for both pairs of group g at token tile ti of chunk."""
                tt = (tcs + 128 * ti) // 128
                psv = psA.tile([128, 256], F32, tag="psA", name="psv")
                for ke in range(KE):
                    mm(psv, xk(ke)[:, 128 * ti:128 * ti + 128],
                       wg[:, ke, 512:768], start=(ke == 0),
                       stop=(ke == KE - 1), skip_group_check=True)
                for pi in range(2):
                    v3 = pairs[2 * g + pi][2]
                    cp = (nc.vector.tensor_copy if ti % 2 == 0
                          else nc.scalar.copy)
                    cp(v3[:, tt, 0:64], psv[:, 128 * pi:128 * pi + 64])
                    cp(v3[:, tt, 65:129],
                       psv[:, 128 * pi + 64:128 * pi + 128])

            def attn_gen(p, qq, blk=4, tail_cb=None):
                """Attention for pair p, q-chunk qq; yields every blk kts."""
                qT, kT, v3, yT = pairs[p]
                qb = 512 * qq
                last = 4 * qq + 3
                psy0 = psY.tile([128, 512], F32, tag="psY", name="psy0")
                psy1 = psY.tile([128, 512], F32, tag="psY", name="psy1")
                psy = (psy0, psy1)

                def pv(item):
                    kt, pt, lo = item
                    for hl in range(2):
                        mm(psy[hl][0:65, lo:512],
                           v3[:, kt, 65 * hl:65 * hl + 65],
                           pt[:, 512 * hl + lo:512 * hl + 512],
                           start=(kt == 0), stop=(kt == last),
                           skip_group_check=True)

                pending = []   # PV runs 2 kts behind scores: its operands
                for kt in range(last + 1):   # are always ready, so the PE
                    # stream stays continuous (avoids HAM re-throttle)
                    col_lo = max(qb, 128 * kt) - qb
                    mm_lo = 256 if col_lo == 384 else col_lo
                    pS = psS.tile([128, 1024], F32, tag="psS")
                    for hl in range(2):
                        hr = 64 * hl
                        mm(pS[:, 512 * hl + mm_lo:512 * hl + 512],
                           kT[hr:hr + 64, 128 * kt:128 * kt + 128],
                           qT[hr:hr + 64, qb + mm_lo:qb + 512],
                           start=True, stop=True, skip_group_check=True)
                    pt = pp.tile([128, 1024], F32R, tag="pt")
                    pS2 = pS.rearrange("p (h c) -> p h c", h=2)
                    pt2 = pt.rearrange("p (h c) -> p h c", h=2)
                    if mm_lo != col_lo:
                        # widened PV reads [256:384] — must be zeros
                        nc.gpsimd.memset(pt2[:, :, 256:384].bitcast(F32), 0)
                    nc.scalar.activation(
                        pt2[:, :, col_lo:512], pS2[:, :, col_lo:512],
                        EXP, scale=0.125)
                    if 128 * kt >= qb:  # diagonal block, both heads
                        # split across DVE and Pool so the two mask muls
                        # run in parallel (PV waits on both)
                        o = col_lo
                        nc.vector.tensor_mul(
                            pt[:, o:o + 128],
                            pt[:, o:o + 128].bitcast(F32), c_mask)
                        o = 512 + col_lo
                        nc.gpsimd.tensor_mul(
                            pt[:, o:o + 128],
                            pt[:, o:o + 128].bitcast(F32), c_mask)
                    pending.append((kt, pt, mm_lo))
                    if len(pending) > 2:
                        pv(pending.pop(0))
                    if kt % blk == 1 and kt != last:
                        yield
                for item in pending:
                    pv(item)
                # ---- softmax drain ----
                # both psum copies first (scalar || DVE) so the psY banks
                # free promptly for the next attention chunk
                ys = (next_ysb(), next_ysb())
                nc.scalar.copy(ys[0][0:65, :], psy[0][0:65, :])
                nc.vector.tensor_copy(ys[1][0:65, :], psy[1][0:65, :])

                def drain_cols(c0, c1):
                    w = c1 - c0
                    for hl in range(2):
                        y_sb = ys[hl]
                        nc.vector.tensor_copy(y_sb[96:97, c0:c1],
                                              y_sb[64:65, c0:c1])
                        bcr = bcp.tile([64, w], F32, tag="bcr")
                        nc.vector.stream_shuffle(bcr, y_sb[64:128, c0:c1],
                                                 BCAST_MASK)
                        bc = bcp.tile([64, w], F32, tag="bc")
                        nc.vector.reciprocal_approx_fast(bc, bcr)
                        col = slice(qb + c0, qb + c1)
                        if hl == 0:
                            nc.gpsimd.tensor_mul(yT[0:64, col],
                                                 y_sb[0:64, c0:c1], bc)
                        else:
                            nc.vector.tensor_mul(yT[64:128, col],
                                                 y_sb[0:64, c0:c1], bc)

                if tail_cb is None:
                    drain_cols(0, 512)
                else:
                    tail_cb(drain_cols)

            def emit_proj(qq, c0=0, c1=512, scalar_only=False):
                qb = 512 * qq
                w = c1 - c0
                for mo in range(E // 128):
                    po = psA.tile([128, w], F32, tag="psA", name="po")
                    for kp in range(NPAIR):
                        mm(po, c_wp[:, kp, 128 * mo:128 * mo + 128],
                           pairs[kp][3][:, qb + c0:qb + c1],
                           start=(kp == 0), stop=(kp == NPAIR - 1),
                           skip_group_check=True)
                    ost = outp.tile([128, w], F32, tag="ost")
                    (nc.scalar.copy if scalar_only or mo % 2
                     else nc.vector.tensor_copy)(ost, po)
                    nc.sync.dma_start(
                        out=outT[128 * mo:128 * mo + 128,
                                 qb + c0:qb + c1],
                        in_=ost)

            def drive(gen, fillers):
                fi = 0
                for _ in gen:
                    if fi < len(fillers):
                        fillers[fi]()
                        fi += 1
                while fi < len(fillers):
                    fillers[fi]()
                    fi += 1

            def chunk_fillers(wg, xt, tcs, g):
                """Section/v fillers for a 512-col chunk of group g."""
                xk = make_xk(xt)
                qk = [pairs[2 * g], pairs[2 * g + 1]]
                return [
                    lambda: emit_section(wg, xk, tcs, 512, 0, 0, qk[0][0]),
                    lambda: emit_section(wg, xk, tcs, 512, 256, 0, qk[0][1]),
                    lambda: (emit_vtile(wg, xk, tcs, 0, g),
                             emit_vtile(wg, xk, tcs, 1, g)),
                    lambda: (emit_vtile(wg, xk, tcs, 2, g),
                             emit_vtile(wg, xk, tcs, 3, g)),
                    lambda: emit_section(wg, xk, tcs, 512, 0, 1, qk[1][0]),
                    lambda: emit_section(wg, xk, tcs, 512, 256, 1, qk[1][1]),
                ]

            # ---------------- schedule ----------------

            alloc_pair(0)
            alloc_pair(1)

            # startup DMAs: first-needed bytes first, fine-grained
            wg0 = wgp.tile([128, KE, 768], F32R, tag="wg", name="wg0")
            nc.sync.dma_start(out=wg0[:, 0:4, 0:256], in_=wq_r[:, 0:4, 0:256])
            x_a = load_x(0, 256)
            nc.sync.dma_start(out=wg0[:, 4:8, 0:256], in_=wq_r[:, 4:8, 0:256])
            nc.sync.dma_start(out=c_cos[:, 0:256], in_=cos2[:, 0:256])
            nc.sync.dma_start(out=c_ssp[:, 0:256], in_=ssp2[:, 0:256])
            nc.sync.dma_start(out=wg0[:, 0:4, 256:512],
                              in_=wk_r[:, 0:4, 0:256])
            nc.sync.dma_start(out=wg0[:, 4:8, 256:512],
                              in_=wk_r[:, 4:8, 0:256])
            x_b = load_x(256, 256)
            nc.sync.dma_start(out=c_cos[:, 256:512], in_=cos2[:, 256:512])
            nc.sync.dma_start(out=c_ssp[:, 256:512], in_=ssp2[:, 256:512])
            nc.sync.dma_start(out=wg0[:, :, 512:768], in_=wv_r[:, :, 0:256])
            nc.sync.dma_start(out=c_mask, in_=maskd)
            for cc in range(1, 4):
                nc.sync.dma_start(out=c_cos[:, 512 * cc:512 * cc + 512],
                                  in_=cos2[:, 512 * cc:512 * cc + 512])
                nc.sync.dma_start(out=c_ssp[:, 512 * cc:512 * cc + 512],
                                  in_=ssp2[:, 512 * cc:512 * cc + 512])

            # startup chunks (group 0, 256+256): q sections first (wq is
            # in flight first), then k, then v
            for tcs, xt in ((0, x_a), (256, x_b)):
                xk = make_xk(xt)
                emit_section(wg0, xk, tcs, 256, 0, 0, pairs[0][0])
                emit_section(wg0, xk, tcs, 256, 0, 1, pairs[1][0])
                emit_section(wg0, xk, tcs, 256, 256, 0, pairs[0][1])
                emit_section(wg0, xk, tcs, 256, 256, 1, pairs[1][1])
                emit_vtile(wg0, xk, tcs, 0, 0)
                emit_vtile(wg0, xk, tcs, 1, 0)

            # rounds 0-2: attention(g0, qq) x qkv(g0, chunk qq+1)
            for qq in range(3):
                tcs = 512 * (qq + 1)
                xt = load_x(tcs, 512)
                f = chunk_fillers(wg0, xt, tcs, 0)
                drive(attn_gen(0, qq), f[:3])
                drive(attn_gen(1, qq), f[3:])

            # group-1 prefetch (WAR on wg0 clears: all g0 qkv mms emitted)
            wg1 = wgp.tile([128, KE, 768], F32R, tag="wg", name="wg1")
            nc.sync.dma_start(out=wg1[:, :, 0:256], in_=wq_r[:, :, 256:512])
            nc.sync.dma_start(out=wg1[:, :, 512:768],
                              in_=wv_r[:, :, 256:512])
            x_g1 = load_x(0, 512)
            nc.sync.dma_start(out=wg1[:, :, 256:512],
                              in_=wk_r[:, :, 256:512])
            nc.sync.dma_start(out=c_wp, in_=wp_r)
            alloc_pair(2)   # 3rd buffer in vp — no WAR with live g0 reads

            # round 3: attention(g0, 3); g1-chunk0 qkv must stay out of
            # gen0's zone (its writes WAR-target buffers gen0 still reads)
            xk1 = make_xk(x_g1)
            drive(attn_gen(0, 3), [])
            drive(attn_gen(1, 3), [
                lambda: emit_section(wg1, xk1, 0, 512, 0, 0, pairs[2][0]),
                lambda: emit_section(wg1, xk1, 0, 512, 256, 0, pairs[2][1]),
            ])
            alloc_pair(3)
            emit_section(wg1, xk1, 0, 512, 0, 1, pairs[3][0])
            emit_section(wg1, xk1, 0, 512, 256, 1, pairs[3][1])
            for ti in range(4):
                emit_vtile(wg1, xk1, 0, ti, 1)

            # rounds 4-7: attention(g1, qq) x qkv(g1, chunk qq+1) + proj
            for qq in range(4):
                if qq < 3:
                    tcs = 512 * (qq + 1)
                    xt = load_x(tcs, 512)
                    f = chunk_fillers(wg1, xt, tcs, 1)
                    drive(attn_gen(2, qq), f[:3])
                    drive(attn_gen(3, qq), f[3:])
                    if qq < 2:
                        emit_proj(qq)
                else:
                    drive(attn_gen(2, 3), [])

                    def tail(drain_cols):
                        # interleave the last drain with the last proj so
                        # the final output DMA starts ~3us earlier; all
                        # proj copies on scalar (DVE is busy draining)
                        drain_cols(0, 256)
                        emit_proj(3, 0, 256, scalar_only=True)
                        drain_cols(256, 512)
                        emit_proj(3, 256, 512, scalar_only=True)

                    drive(attn_gen(3, 3, tail_cb=tail),
                          [lambda: emit_proj(2, scalar_only=True)])

    nc.compile()
    _NC_CACHE["nc"] = nc
    return nc


def _host_tables():
    inv_freq = 1.0 / (10000.0 ** (np.arange(0, D, 2, dtype=np.float32) / D))
    t = np.arange(T, dtype=np.float32)
    freqs = np.outer(t, inv_freq)                     # [T, 32]
    emb = np.concatenate([freqs, freqs], -1)          # [T, 64]
    cos_t = np.cos(emb).T.astype(np.float32)          # [64, T]
    sin_t = np.sin(emb).T.astype(np.float32)
    # rope(x)[d] = x[d]*cos[d] + x[d^1]*ssin[d],
    #   ssin[2i] = -sin[2i], ssin[2i+1] = +sin[2i+1]
    # device computes swap(x * ssp) with swap[d] = d^1, so ssp[d] = ssin[d^1]
    ssp = np.empty_like(sin_t)
    ssp[0::2] = sin_t[1::2]       # even d: +sin(emb[d+1])
    ssp[1::2] = -sin_t[0::2]      # odd d:  -sin(emb[d-1])
    cos2 = np.concatenate([cos_t, cos_t], 0)          # [128, T]
    ssp2 = np.concatenate([ssp, ssp], 0)
    r = np.arange(128)
    maskd = (r[:, None] <= r[None, :]).astype(np.float32)
    return cos2, ssp2, maskd


def _core_inputs(x, w_attn, w_proj, c):
    cos2, ssp2, maskd = _host_tables()
    b, g = c // 2, c % 2
    j0 = g * F
    return {
        "xT": np.ascontiguousarray(x[b].T),
        "wqT": np.ascontiguousarray(w_attn[j0:j0 + F].T),
        "wkT": np.ascontiguousarray(w_attn[E + j0:E + j0 + F].T),
        "wvT": np.ascontiguousarray(w_attn[2 * E + j0:2 * E + j0 + F].T),
        "wpT": np.ascontiguousarray(w_proj[:, j0:j0 + F].T),
        "cos2": cos2, "ssp2": ssp2, "maskd": maskd,
        "ones16": np.ones((128, 16), dtype=np.float32),
    }


def kernel(x, w_attn, w_proj):
    x = np.asarray(x, dtype=np.float32)
    w_attn = np.asarray(w_attn, dtype=np.float32)
    w_proj = np.asarray(w_proj, dtype=np.float32)

    nc = _build_program()
    in_maps = [_core_inputs(x, w_attn, w_proj, c) for c in range(NCORES)]
    res = run_bass_kernel_spmd(nc, in_maps, core_ids=list(range(NCORES)))
    out = np.empty((B, T, E), dtype=np.float32)
    for b in range(B):
        acc = res.results[2 * b]["outT"] + res.results[2 * b + 1]["outT"]
        out[b] = acc.T
    return out


# revision 28
# speedup vs baseline: 1.1430x; 1.0100x over previous
"""Causal self-attention (B=4, T=2048, E=1024, H=16, D=64) on 8 trn2 cores.

Sharding: core c -> (batch b = c//2, head-group g = c%2 of 8 heads).
Each core computes qkv projection + RoPE + causal attention + its partial
output projection for its (batch, head-group); host sums the two
head-group partials per batch and transposes back.

Device data layout is feature-major ("T" suffix = [features, tokens]):
scores are computed k-major (S.T blocks [tk=128, tq]) so causal masking
skips ~half the matmuls, and softmax normalization comes from an extra
ones-column in the v operand of the PV matmul (the denominator lands in
one PSUM partition row at zero extra matmul cost).

All matmuls run in float32r (full PE rate for N>=256); diagonal blocks
that would fall to N=128 (4 cyc/row in fp32r) are widened to N=256 —
the extra score columns land in unread PSUM, and the extra PV columns
read zeros memset into pt.

v5 scheduling: the program is emitted as software-pipelined "rounds".
Round(qq) interleaves, at 3-kt granularity, the attention of q-chunk qq
(scores/exp/mask/PV on PE+Act+Pool) with the qkv+RoPE sections of the
NEXT chunk (PE+DVE+Pool) and the output projection of a completed
q-chunk, so the PE never drains its pipeline while the activation
engine catches up on exp. Elementwise work is spread across all three
non-PE engines:
  DVE:  rope muls (PSUM drain) + swap shuffle, drain shuffle/recip,
        hl1 drain mul (cross-partition write), half the psum copies
  Pool: rope adds, causal mask muls, hl0 drain mul, pt memsets
  Act:  exp, psy drain copies, half the psum copies
The drain broadcasts the denominator row with ONE 64-row stream_shuffle
(denominator duplicated to partition 96 first), and y_sb pool buffers
are memset once at startup so the shuffle never reads uninitialized
SBUF (keeps CoreSim clean).
"""
import sys

sys.path.insert(0, "/opt/trn_rl_repo")

from contextlib import ExitStack

import numpy as np

import concourse.bass as bass
import concourse.bacc as bacc
import concourse.tile as tile
from concourse import mybir
from concourse.bass_utils import run_bass_kernel_spmd

B, T, E, H, D = 4, 2048, 1024, 16, 64
NCORES = 8
HG = H // 2          # heads per shard (8)
F = HG * D           # features per shard (512)
NPAIR = F // 128     # head pairs per shard (4)
NGRP = NPAIR // 2    # pair groups (2)
KE = E // 128        # contraction tiles over E (8)
NKT = T // 128       # k tiles (16)
F32 = mybir.dt.float32
F32R = mybir.dt.float32r
EXP = mybir.ActivationFunctionType.Exp
SWAP_MASK = [i ^ 1 for i in range(32)]   # rope pair swap within banks
BCAST_MASK = [0] * 32                     # all lanes take lane 0

_NC_CACHE = {}


def _build_program():
    if "nc" in _NC_CACHE:
        return _NC_CACHE["nc"]
    nc = bacc.Bacc("TRN2", target_bir_lowering=False, debug=False,
                   num_devices=NCORES)
    mm = nc.tensor.matmul
    xT = nc.dram_tensor("xT", [E, T], F32R, kind="ExternalInput").ap()
    wqT = nc.dram_tensor("wqT", [E, F], F32R, kind="ExternalInput").ap()
    wkT = nc.dram_tensor("wkT", [E, F], F32R, kind="ExternalInput").ap()
    wvT = nc.dram_tensor("wvT", [E, F], F32R, kind="ExternalInput").ap()
    wpT = nc.dram_tensor("wpT", [F, E], F32R, kind="ExternalInput").ap()
    cos2 = nc.dram_tensor("cos2", [128, T], F32, kind="ExternalInput").ap()
    ssp2 = nc.dram_tensor("ssp2", [128, T], F32, kind="ExternalInput").ap()
    maskd = nc.dram_tensor("maskd", [128, 128], F32, kind="ExternalInput").ap()
    ones16 = nc.dram_tensor("ones16", [128, NKT], F32R,
                            kind="ExternalInput").ap()
    outT = nc.dram_tensor("outT", [E, T], F32, kind="ExternalOutput").ap()

    xT_r = xT.rearrange("(ke p) t -> p ke t", p=128)
    wq_r = wqT.rearrange("(ke p) j -> p ke j", p=128)
    wk_r = wkT.rearrange("(ke p) j -> p ke j", p=128)
    wv_r = wvT.rearrange("(ke p) j -> p ke j", p=128)
    wp_r = wpT.rearrange("(kf p) o -> p kf o", p=128)

    with tile.TileContext(nc) as tc:
        with ExitStack() as ctx:
            const = ctx.enter_context(tc.tile_pool(name="const", bufs=1))
            wgp = ctx.enter_context(tc.tile_pool(name="wgp", bufs=1))
            xp = ctx.enter_context(tc.tile_pool(name="xp", bufs=2))
            qkp = ctx.enter_context(tc.tile_pool(name="qkp", bufs=2))
            vp = ctx.enter_context(tc.tile_pool(name="vp", bufs=3))
            yp = ctx.enter_context(tc.tile_pool(name="yp", bufs=4))
            pp = ctx.enter_context(tc.tile_pool(name="pp", bufs=4))
            tmp = ctx.enter_context(tc.tile_pool(name="tmp", bufs=2))
            ysbp = ctx.enter_context(tc.tile_pool(name="ysbp", bufs=2))
            bcp = ctx.enter_context(tc.tile_pool(name="bcp", bufs=2))
            outp = ctx.enter_context(tc.tile_pool(name="outp", bufs=3))
            psA = ctx.enter_context(
                tc.tile_pool(name="psA", bufs=2, space="PSUM"))
            psS = ctx.enter_context(
                tc.tile_pool(name="psS", bufs=2, space="PSUM"))
            psY = ctx.enter_context(
                tc.tile_pool(name="psY", bufs=2, space="PSUM"))

            c_cos = const.tile([128, T], F32, tag="cos")
            c_ssp = const.tile([128, T], F32, tag="ssp")
            c_mask = const.tile([128, 128], F32, tag="mask")
            c_wp = const.tile([128, NPAIR, E], F32R, tag="wp")

            # y_sb drain buffers: persistent tiles, reused round-robin by
            # the drains (subtile WAR deps order the reuse). Rows 65:96 and
            # 97:128 are read by the 64-row broadcast shuffle but never
            # written per-drain — memset them once here.
            ysb_tiles = []
            for i in range(2):
                t = ysbp.tile([128, 512], F32, tag="ysb", name=f"ysb{i}")
                nc.vector.memset(t[64:96, :], 0)
                nc.vector.memset(t[96:128, :], 0)
                ysb_tiles.append(t)
            ysb_ctr = [0]

            def next_ysb():
                t = ysb_tiles[ysb_ctr[0] % 2]
                ysb_ctr[0] += 1
                return t

            pairs = [None] * NPAIR   # (qT, kT, v3, yT) per global pair

            def alloc_pair(p, with_ones=True):
                qT = qkp.tile([128, T], F32R, tag="qT", name=f"qT{p}")
                kT = qkp.tile([128, T], F32R, tag="kT", name=f"kT{p}")
                v3 = vp.tile([128, NKT, 130], F32R, tag="v3", name=f"v3{p}")
                yT = yp.tile([128, T], F32R, tag="yT", name=f"yT{p}")
                pairs[p] = (qT, kT, v3, yT)
                if with_ones:
                    emit_ones(p)

            def emit_ones(p):
                v3 = pairs[p][2]
                nc.sync.dma_start(out=v3[:, :, 64], in_=ones16)
                nc.sync.dma_start(out=v3[:, :, 129], in_=ones16)

            def load_x(tcs, tch):
                xca = xp.tile([128, KE // 2, tch], F32R, tag="xc",
                              name=f"xca{tcs}")
                xcb = xp.tile([128, KE // 2, tch], F32R, tag="xc",
                              name=f"xcb{tcs}")
                nc.sync.dma_start(out=xca,
                                  in_=xT_r[:, 0:KE // 2, tcs:tcs + tch])
                nc.sync.dma_start(out=xcb,
                                  in_=xT_r[:, KE // 2:KE, tcs:tcs + tch])
                return xca, xcb

            def make_xk(xt):
                xca, xcb = xt
                return lambda ke: (xca if ke < KE // 2
                                   else xcb)[:, ke % (KE // 2), :]

            # ---------------- emit helpers ----------------

            def emit_section(wg, xk, tcs, tch, sec, pi, dst):
                """One 128-feature q or k section: 8 matmuls + rope."""
                tcol = slice(tcs, tcs + tch)
                ps = psA.tile([128, tch], F32, tag="psA", name="ps")
                wcol = sec + 128 * pi
                for ke in range(KE):
                    mm(ps, wg[:, ke, wcol:wcol + 128], xk(ke),
                       start=(ke == 0), stop=(ke == KE - 1),
                       skip_group_check=True)
                # rope: dst = ps*cos + swap(ps*ssp); psA freed by the 2 muls
                bt0 = tmp.tile([128, tch], F32, tag="bt0", bufs=1)
                nc.vector.tensor_mul(bt0, ps, c_ssp[:, tcol])
                ct = tmp.tile([128, tch], F32, tag="ct", bufs=2)
                nc.vector.tensor_mul(ct, ps, c_cos[:, tcol])
                bt = tmp.tile([128, tch], F32, tag="bt", bufs=2)
                nc.vector.stream_shuffle(bt, bt0, SWAP_MASK)
                nc.gpsimd.tensor_add(dst[:, tcol], ct, bt)

            def emit_vtile(wg, xk, tcs, ti, g):
                """v for both pairs of group g at token tile ti of chunk."""
                tt = (tcs + 128 * ti) // 128
                psv = psA.tile([128, 256], F32, tag="psA", name="psv")
                for ke in range(KE):
                    mm(psv, xk(ke)[:, 128 * ti:128 * ti + 128],
                       wg[:, ke, 512:768], start=(ke == 0),
                       stop=(ke == KE - 1), skip_group_check=True)
                for pi in range(2):
                    v3 = pairs[2 * g + pi][2]
                    cp = (nc.vector.tensor_copy if ti % 2 == 0
                          else nc.scalar.copy)
                    cp(v3[:, tt, 0:64], psv[:, 128 * pi:128 * pi + 64])
                    cp(v3[:, tt, 65:129],
                       psv[:, 128 * pi + 64:128 * pi + 128])

            def attn_gen(p, qq, blk=4, tail_cb=None):
                """Attention for pair p, q-chunk qq; yields every blk kts."""
                qT, kT, v3, yT = pairs[p]
                qb = 512 * qq
                last = 4 * qq + 3
                psy0 = psY.tile([128, 512], F32, tag="psY", name="psy0")
                psy1 = psY.tile([128, 512], F32, tag="psY", name="psy1")
                psy = (psy0, psy1)

                def pv(item):
                    kt, pt, lo = item
                    for hl in range(2):
                        mm(psy[hl][0:65, lo:512],
                           v3[:, kt, 65 * hl:65 * hl + 65],
                           pt[:, 512 * hl + lo:512 * hl + 512],
                           start=(kt == 0), stop=(kt == last),
                           skip_group_check=True)

                pending = []   # PV runs 2 kts behind scores: its operands
                for kt in range(last + 1):   # are always ready, so the PE
                    # stream stays continuous (avoids HAM re-throttle)
                    col_lo = max(qb, 128 * kt) - qb
                    mm_lo = 256 if col_lo == 384 else col_lo
                    pS = psS.tile([128, 1024], F32, tag="psS")
                    for hl in range(2):
                        hr = 64 * hl
                        mm(pS[:, 512 * hl + mm_lo:512 * hl + 512],
                           kT[hr:hr + 64, 128 * kt:128 * kt + 128],
                           qT[hr:hr + 64, qb + mm_lo:qb + 512],
                           start=True, stop=True, skip_group_check=True)
                    pt = pp.tile([128, 1024], F32R, tag="pt")
                    pS2 = pS.rearrange("p (h c) -> p h c", h=2)
                    pt2 = pt.rearrange("p (h c) -> p h c", h=2)
                    if mm_lo != col_lo:
                        # widened PV reads [256:384] — must be zeros
                        nc.gpsimd.memset(pt2[:, :, 256:384].bitcast(F32), 0)
                    nc.scalar.activation(
                        pt2[:, :, col_lo:512], pS2[:, :, col_lo:512],
                        EXP, scale=0.125)
                    if 128 * kt >= qb:  # diagonal block, both heads
                        # split across DVE and Pool so the two mask muls
                        # run in parallel (PV waits on both)
                        o = col_lo
                        nc.vector.tensor_mul(
                            pt[:, o:o + 128],
                            pt[:, o:o + 128].bitcast(F32), c_mask)
                        o = 512 + col_lo
                        nc.gpsimd.tensor_mul(
                            pt[:, o:o + 128],
                            pt[:, o:o + 128].bitcast(F32), c_mask)
                    pending.append((kt, pt, mm_lo))
                    if len(pending) > 2:
                        pv(pending.pop(0))
                    if kt % blk == 1 and kt != last:
                        yield
                for item in pending:
                    pv(item)
                # ---- softmax drain ----
                # both psum copies first (scalar || DVE) so the psY banks
                # free promptly for the next attention chunk
                ys = (next_ysb(), next_ysb())
                nc.scalar.copy(ys[0][0:65, :], psy[0][0:65, :])
                nc.vector.tensor_copy(ys[1][0:65, :], psy[1][0:65, :])

                def drain_cols(c0, c1):
                    w = c1 - c0
                    for hl in range(2):
                        y_sb = ys[hl]
                        nc.vector.tensor_copy(y_sb[96:97, c0:c1],
                                              y_sb[64:65, c0:c1])
                        bcr = bcp.tile([64, w], F32, tag="bcr")
                        nc.vector.stream_shuffle(bcr, y_sb[64:128, c0:c1],
                                                 BCAST_MASK)
                        bc = bcp.tile([64, w], F32, tag="bc")
                        nc.vector.reciprocal_approx_fast(bc, bcr)
                        col = slice(qb + c0, qb + c1)
                        if hl == 0:
                            nc.gpsimd.tensor_mul(yT[0:64, col],
                                                 y_sb[0:64, c0:c1], bc)
                        else:
                            nc.vector.tensor_mul(yT[64:128, col],
                                                 y_sb[0:64, c0:c1], bc)

                if tail_cb is None:
                    drain_cols(0, 512)
                else:
                    tail_cb(drain_cols)

            def emit_proj(qq, c0=0, c1=512, scalar_only=False):
                qb = 512 * qq
                w = c1 - c0
                for mo in range(E // 128):
                    po = psA.tile([128, w], F32, tag="psA", name="po")
                    for kp in range(NPAIR):
                        mm(po, c_wp[:, kp, 128 * mo:128 * mo + 128],
                           pairs[kp][3][:, qb + c0:qb + c1],
                           start=(kp == 0), stop=(kp == NPAIR - 1),
                           skip_group_check=True)
                    ost = outp.tile([128, w], F32, tag="ost")
                    (nc.scalar.copy if scalar_only or mo % 2
                     else nc.vector.tensor_copy)(ost, po)
                    nc.sync.dma_start(
                        out=outT[128 * mo:128 * mo + 128,
                                 qb + c0:qb + c1],
                        in_=ost)

            def drive(gen, fillers):
                fi = 0
                for _ in gen:
                    if fi < len(fillers):
                        fillers[fi]()
                        fi += 1
                while fi < len(fillers):
                    fillers[fi]()
                    fi += 1

            def chunk_fillers(wg, xt, tcs, g):
                """Section/v fillers for a 512-col chunk of group g."""
                xk = make_xk(xt)
                qk = [pairs[2 * g], pairs[2 * g + 1]]
                return [
                    lambda: emit_section(wg, xk, tcs, 512, 0, 0, qk[0][0]),
                    lambda: emit_section(wg, xk, tcs, 512, 256, 0, qk[0][1]),
                    lambda: (emit_vtile(wg, xk, tcs, 0, g),
                             emit_vtile(wg, xk, tcs, 1, g)),
                    lambda: (emit_vtile(wg, xk, tcs, 2, g),
                             emit_vtile(wg, xk, tcs, 3, g)),
                    lambda: emit_section(wg, xk, tcs, 512, 0, 1, qk[1][0]),
                    lambda: emit_section(wg, xk, tcs, 512, 256, 1, qk[1][1]),
                ]

            # ---------------- schedule ----------------

            alloc_pair(0)
            alloc_pair(1)

            # startup DMAs: first-needed bytes first, fine-grained
            wg0 = wgp.tile([128, KE, 768], F32R, tag="wg", name="wg0")
            nc.sync.dma_start(out=wg0[:, 0:4, 0:256], in_=wq_r[:, 0:4, 0:256])
            x_a = load_x(0, 256)
            nc.sync.dma_start(out=wg0[:, 4:8, 0:256], in_=wq_r[:, 4:8, 0:256])
            nc.sync.dma_start(out=c_cos[:, 0:256], in_=cos2[:, 0:256])
            nc.sync.dma_start(out=c_ssp[:, 0:256], in_=ssp2[:, 0:256])
            nc.sync.dma_start(out=wg0[:, 0:4, 256:512],
                              in_=wk_r[:, 0:4, 0:256])
            nc.sync.dma_start(out=wg0[:, 4:8, 256:512],
                              in_=wk_r[:, 4:8, 0:256])
            x_b = load_x(256, 256)
            nc.sync.dma_start(out=c_cos[:, 256:512], in_=cos2[:, 256:512])
            nc.sync.dma_start(out=c_ssp[:, 256:512], in_=ssp2[:, 256:512])
            nc.sync.dma_start(out=wg0[:, :, 512:768], in_=wv_r[:, :, 0:256])
            nc.sync.dma_start(out=c_mask, in_=maskd)
            for cc in range(1, 4):
                nc.sync.dma_start(out=c_cos[:, 512 * cc:512 * cc + 512],
                                  in_=cos2[:, 512 * cc:512 * cc + 512])
                nc.sync.dma_start(out=c_ssp[:, 512 * cc:512 * cc + 512],
                                  in_=ssp2[:, 512 * cc:512 * cc + 512])

            # startup chunks (group 0, 256+256): q sections first (wq is
            # in flight first), then k, then v
            for tcs, xt in ((0, x_a), (256, x_b)):
                xk = make_xk(xt)
                emit_section(wg0, xk, tcs, 256, 0, 0, pairs[0][0])
                emit_section(wg0, xk, tcs, 256, 0, 1, pairs[1][0])
                emit_section(wg0, xk, tcs, 256, 256, 0, pairs[0][1])
                emit_section(wg0, xk, tcs, 256, 256, 1, pairs[1][1])
                emit_vtile(wg0, xk, tcs, 0, 0)
                emit_vtile(wg0, xk, tcs, 1, 0)

            # rounds 0-2: attention(g0, qq) x qkv(g0, chunk qq+1)
            for qq in range(3):
                tcs = 512 * (qq + 1)
                xt = load_x(tcs, 512)
                f = chunk_fillers(wg0, xt, tcs, 0)
                drive(attn_gen(0, qq), f[:3])
                drive(attn_gen(1, qq), f[3:])

            # group-1 prefetch (WAR on wg0 clears: all g0 qkv mms emitted)
            wg1 = wgp.tile([128, KE, 768], F32R, tag="wg", name="wg1")
            nc.sync.dma_start(out=wg1[:, :, 0:256], in_=wq_r[:, :, 256:512])
            nc.sync.dma_start(out=wg1[:, :, 512:768],
                              in_=wv_r[:, :, 256:512])
            x_g1 = load_x(0, 512)
            nc.sync.dma_start(out=wg1[:, :, 256:512],
                              in_=wk_r[:, :, 256:512])
            nc.sync.dma_start(out=c_wp, in_=wp_r)
            alloc_pair(2)   # 3rd buffer in vp — no WAR with live g0 reads

            # round 3: attention(g0, 3); g1-chunk0 qkv must stay out of
            # gen0's zone (its writes WAR-target buffers gen0 still reads)
            xk1 = make_xk(x_g1)
            drive(attn_gen(0, 3), [])
            drive(attn_gen(1, 3), [
                lambda: emit_section(wg1, xk1, 0, 512, 0, 0, pairs[2][0]),
                lambda: emit_section(wg1, xk1, 0, 512, 256, 0, pairs[2][1]),
            ])
            alloc_pair(3)
            emit_section(wg1, xk1, 0, 512, 0, 1, pairs[3][0])
            emit_section(wg1, xk1, 0, 512, 256, 1, pairs[3][1])
            for ti in range(4):
                emit_vtile(wg1, xk1, 0, ti, 1)

            # rounds 4-7: attention(g1, qq) x qkv(g1, chunk qq+1) + proj
            for qq in range(4):
                if qq < 3:
                    tcs = 512 * (qq + 1)
                    xt = load_x(tcs, 512)
                    f = chunk_fillers(wg1, xt, tcs, 1)
                    drive(attn_gen(2, qq), f[:3])
                    drive(attn_gen(3, qq), f[3:])
                    if qq < 2:
                        emit_proj(qq)
                else:
                    drive(attn_gen(2, 3), [])

                    def tail(drain_cols):
                        # interleave the last drain with the last proj so
                        # the final output DMA starts ~3us earlier; all
                        # proj copies on scalar (DVE is busy draining)
                        drain_cols(0, 256)
                        emit_proj(3, 0, 256, scalar_only=True)
                        drain_cols(256, 512)
                        emit_proj(3, 256, 512, scalar_only=True)

                    drive(attn_gen(3, 3, tail_cb=tail),
                          [lambda: emit_proj(2, scalar_only=True)])

    nc.compile()
    _NC_CACHE["nc"] = nc
    return nc


def _host_tables():
    inv_freq = 1.0 / (10000.0 ** (np.arange(0, D, 2, dtype=np.float32) / D))
    t = np.arange(T, dtype=np.float32)
    freqs = np.outer(t, inv_freq)                     # [T, 32]
    emb = np.concatenate([freqs, freqs], -1)          # [T, 64]
    cos_t = np.cos(emb).T.astype(np.float32)          # [64, T]
    sin_t = np.sin(emb).T.astype(np.float32)
    # rope(x)[d] = x[d]*cos[d] + x[d^1]*ssin[d],
    #   ssin[2i] = -sin[2i], ssin[2i+1] = +sin[2i+1]
    # device computes swap(x * ssp) with swap[d] = d^1, so ssp[d] = ssin[d^1]
    ssp = np.empty_like(sin_t)
    ssp[0::2] = sin_t[1::2]       # even d: +sin(emb[d+1])
    ssp[1::2] = -sin_t[0::2]      # odd d:  -sin(emb[d-1])
    cos2 = np.concatenate([cos_t, cos_t], 0)          # [128, T]
    ssp2 = np.concatenate([ssp, ssp], 0)
    r = np.arange(128)
    maskd = (r[:, None] <= r[None, :]).astype(np.float32)
    return cos2, ssp2, maskd


def _core_inputs(x, w_attn, w_proj, c):
    cos2, ssp2, maskd = _host_tables()
    b, g = c // 2, c % 2
    j0 = g * F
    return {
        "xT": np.ascontiguousarray(x[b].T),
        "wqT": np.ascontiguousarray(w_attn[j0:j0 + F].T),
        "wkT": np.ascontiguousarray(w_attn[E + j0:E + j0 + F].T),
        "wvT": np.ascontiguousarray(w_attn[2 * E + j0:2 * E + j0 + F].T),
        "wpT": np.ascontiguousarray(w_proj[:, j0:j0 + F].T),
        "cos2": cos2, "ssp2": ssp2, "maskd": maskd,
        "ones16": np.ones((128, 16), dtype=np.float32),
    }


def kernel(x, w_attn, w_proj):
    x = np.asarray(x, dtype=np.float32)
    w_attn = np.asarray(w_attn, dtype=np.float32)
    w_proj = np.asarray(w_proj, dtype=np.float32)

    nc = _build_program()
    in_maps = [_core_inputs(x, w_attn, w_proj, c) for c in range(NCORES)]
    res = run_bass_kernel_spmd(nc, in_maps, core_ids=list(range(NCORES)))
    out = np.empty((B, T, E), dtype=np.float32)
    for b in range(B):
        acc = res.results[2 * b]["outT"] + res.results[2 * b + 1]["outT"]
        out[b] = acc.T
    return out
